# revision 4
# baseline (speedup 1.0000x reference)
"""MoE layer (8 experts, top-2, shared expert) on 8 Trainium2 cores.

Sharding: expert-parallel. Core c holds expert c's gate/up/down weights and
a 1/8 tensor-parallel shard (256 cols) of the shared FFN. x and the router
are replicated; every core computes the full router (exact fp32) and its
expert's SwiGLU densely over all tokens, scales by its combine column, adds
its shared-FFN partial, and returns a [T, D] partial. Host sums the 8
partials — that sum is exactly routed + shared of the reference.

Expert/shared matmuls run in f32r (full PE rate, ~1.5e-4 rel err); the
router runs in true fp32 from untransposed-precision x because the minimum
top2-vs-top3 logit gap of the workload (~3e-4) is too small for f32r noise.
"""

import numpy as np
from contextlib import ExitStack

import concourse.bass as bass
import concourse.tile as tile
from concourse import bacc, mybir
from concourse.bass_utils import run_bass_kernel_spmd
from concourse.masks import make_identity

T, D, E = 2048, 1024, 8
F = 512          # per-expert FFN width
FS = 256         # shared FFN width per core (2048 / 8)
P = 128
NCORES = 8

TT = T // P      # 16 token tiles
DC = D // P      # 8 contraction chunks
FC = F // P      # 4 expert-f chunks
SC = FS // P     # 2 shared-f chunks
NTC = T // 512   # 4 token chunks of 512

DT = mybir.dt.float32
DTR = mybir.dt.float32r
AF = mybir.ActivationFunctionType
ALU = mybir.AluOpType
AX = mybir.AxisListType

_NC_CACHE = None


def _build_nc():
    nc = bacc.Bacc("TRN2", target_bir_lowering=False, debug=False,
                   num_devices=NCORES)
    x = nc.dram_tensor("x", [T, D], DT, kind="ExternalInput")
    rw = nc.dram_tensor("rw", [D, E], DT, kind="ExternalInput")
    wg = nc.dram_tensor("wg", [D, F], DT, kind="ExternalInput")
    wu = nc.dram_tensor("wu", [D, F], DT, kind="ExternalInput")
    wd = nc.dram_tensor("wd", [F, D], DT, kind="ExternalInput")
    sg = nc.dram_tensor("sg", [D, FS], DT, kind="ExternalInput")
    su = nc.dram_tensor("su", [D, FS], DT, kind="ExternalInput")
    sd = nc.dram_tensor("sd", [FS, D], DT, kind="ExternalInput")
    esel = nc.dram_tensor("esel", [P, E], DT, kind="ExternalInput")
    out = nc.dram_tensor("out", [T, D], DT, kind="ExternalOutput")

    with tile.TileContext(nc) as tc, ExitStack() as ctx:
        const = ctx.enter_context(tc.tile_pool(name="const", bufs=1))
        ident = const.tile([P, P], DT)
        make_identity(nc, ident[:])
        esel_sb = const.tile([P, E], DT)
        nc.sync.dma_start(esel_sb[:], esel[:])
        rw_sb = const.tile([P, DC, E], DT)
        nc.sync.dma_start(rw_sb[:], rw.rearrange("(c p) e -> p c e", p=P))

        wgt = ctx.enter_context(tc.tile_pool(name="wgt", bufs=1))
        wg_sb = wgt.tile([P, DC, F], DTR)
        nc.sync.dma_start(wg_sb[:], wg.rearrange("(c p) f -> p c f", p=P).bitcast(DTR))
        wu_sb = wgt.tile([P, DC, F], DTR)
        nc.sync.dma_start(wu_sb[:], wu.rearrange("(c p) f -> p c f", p=P).bitcast(DTR))
        wd_sb = wgt.tile([P, FC, D], DTR)
        nc.sync.dma_start(wd_sb[:], wd.rearrange("(c p) d -> p c d", p=P).bitcast(DTR))
        sg_sb = wgt.tile([P, DC, FS], DTR)
        nc.sync.dma_start(sg_sb[:], sg.rearrange("(c p) f -> p c f", p=P).bitcast(DTR))
        su_sb = wgt.tile([P, DC, FS], DTR)
        nc.sync.dma_start(su_sb[:], su.rearrange("(c p) f -> p c f", p=P).bitcast(DTR))
        sd_sb = wgt.tile([P, SC, D], DTR)
        nc.sync.dma_start(sd_sb[:], sd.rearrange("(c p) d -> p c d", p=P).bitcast(DTR))

        big = ctx.enter_context(tc.tile_pool(name="big", bufs=1))
        xT_sb = big.tile([P, DC, T], DTR)   # transposed x, f32r-rounded
        cmb_sb = big.tile([P, TT], DT)      # combine weight column per token tile

        # ---- Phase A: transpose x, exact-fp32 router, combine weights ----
        with tc.tile_pool(name="pha", bufs=2) as pha, \
             tc.tile_pool(name="pha3", bufs=3) as pha3, \
             tc.tile_pool(name="rtr", bufs=2) as rtr, \
             tc.tile_pool(name="ps_a", bufs=3, space="PSUM") as ps_a, \
             tc.tile_pool(name="ps_r", bufs=2, space="PSUM") as ps_r:
          for tt in range(TT):
            x_sb = pha3.tile([P, D], DT, tag="x_in")
            nc.sync.dma_start(x_sb[:], x[tt * P:(tt + 1) * P, :])
            xrt = rtr.tile([P, DC, P], DT, tag="xrt")  # fp32 xT for the router
            for g in range(2):  # two groups of 4 d-chunks per psum bank
                ps_tr = ps_a.tile([P, 4, P], DT, tag="tr")
                for j in range(4):
                    dc = g * 4 + j
                    nc.tensor.transpose(ps_tr[:, j], x_sb[:, dc * P:(dc + 1) * P],
                                        ident[:])
                # f32r copy feeds the expert/shared matmuls
                nc.scalar.copy(
                    xT_sb[:, g * 4:(g + 1) * 4, tt * P:(tt + 1) * P].bitcast(DTR),
                    ps_tr[:])
                # exact fp32 copy feeds the router
                nc.vector.tensor_copy(xrt[:, g * 4:(g + 1) * 4], ps_tr[:])

            ps_lg = ps_r.tile([P, E], DT, tag="lg")
            for dc in range(DC):
                nc.tensor.matmul(ps_lg[:], xrt[:, dc], rw_sb[:, dc],
                                 start=(dc == 0), stop=(dc == DC - 1))

            # top-2-of-8 softmax weights, renormalized; column e via esel
            m1 = pha.tile([P, 1], DT, tag="m1")
            nc.vector.reduce_max(out=m1[:], in_=ps_lg[:], axis=AX.X)
            nm1 = pha.tile([P, 1], DT, tag="nm1")
            nc.vector.tensor_scalar_mul(nm1[:], m1[:], -1.0)
            p_sb = pha.tile([P, E], DT, tag="p")
            nc.scalar.activation(p_sb[:], ps_lg[:], AF.Exp, bias=nm1[:])
            is1 = pha.tile([P, E], DT, tag="is1")
            nc.vector.tensor_scalar(is1[:], p_sb[:], 1.0, None, op0=ALU.is_ge)
            pm = pha.tile([P, E], DT, tag="pm")
            nc.vector.tensor_sub(pm[:], p_sb[:], is1[:])
            m2 = pha.tile([P, 1], DT, tag="m2")
            nc.vector.reduce_max(out=m2[:], in_=pm[:], axis=AX.X)
            s = pha.tile([P, 1], DT, tag="s")
            nc.vector.tensor_scalar_add(s[:], m2[:], 1.0)
            r = pha.tile([P, 1], DT, tag="r")
            nc.vector.reciprocal(r[:], s[:])
            # t1 = (p >= m2 ? 1 : 0) * r ; w = t1 * p ; col = sum(w * esel)
            t1 = pha.tile([P, E], DT, tag="t1")
            nc.vector.tensor_scalar(t1[:], p_sb[:], m2[:], r[:],
                                    op0=ALU.is_ge, op1=ALU.mult)
            w_sb = pha.tile([P, E], DT, tag="w")
            nc.vector.tensor_mul(w_sb[:], t1[:], p_sb[:])
            msk = pha.tile([P, E], DT, tag="msk")
            nc.vector.tensor_mul(msk[:], w_sb[:], esel_sb[:])
            nc.vector.reduce_sum(out=cmb_sb[:, tt:tt + 1], in_=msk[:], axis=AX.X)

        # ---- Phase B: expert SwiGLU + shared FFN shard, per 512-token chunk ----
        act = ctx.enter_context(tc.tile_pool(name="act", bufs=2))
        hpool = ctx.enter_context(tc.tile_pool(name="hpool", bufs=1))
        routp = ctx.enter_context(tc.tile_pool(name="routp", bufs=8))
        outp = ctx.enter_context(tc.tile_pool(name="outp", bufs=3))
        ps_g = ctx.enter_context(tc.tile_pool(name="ps_g", bufs=2, space="PSUM"))
        ps_u = ctx.enter_context(tc.tile_pool(name="ps_u", bufs=2, space="PSUM"))
        ps_y = ctx.enter_context(tc.tile_pool(name="ps_y", bufs=2, space="PSUM"))

        for tc_i in range(NTC):
            tsl = slice(tc_i * 512, (tc_i + 1) * 512)
            routed_tiles = {}
            hT = hpool.tile([P, FC, 512], DTR, tag="hT")
            for fc in range(FC):
                pg = ps_g.tile([P, 512], DT, tag="g")
                pu = ps_u.tile([P, 512], DT, tag="u")
                for dc in range(DC):
                    nc.tensor.matmul(pg[:], wg_sb[:, dc, fc * P:(fc + 1) * P],
                                     xT_sb[:, dc, tsl],
                                     start=(dc == 0), stop=(dc == DC - 1))
                for dc in range(DC):
                    nc.tensor.matmul(pu[:], wu_sb[:, dc, fc * P:(fc + 1) * P],
                                     xT_sb[:, dc, tsl],
                                     start=(dc == 0), stop=(dc == DC - 1))
                sg_act = act.tile([P, 512], DT, tag="silu")
                nc.scalar.activation(sg_act[:], pg[:], AF.Silu)
                nc.vector.tensor_mul(hT[:, fc], sg_act[:], pu[:])

            # expert down + combine-scale
            for j in range(4):
                tt = tc_i * 4 + j
                for dn in range(2):
                    py = ps_y.tile([P, 512], DT, tag="y")
                    for fc in range(FC):
                        nc.tensor.matmul(py[:], hT[:, fc, j * P:(j + 1) * P],
                                         wd_sb[:, fc, dn * 512:(dn + 1) * 512],
                                         start=(fc == 0), stop=(fc == FC - 1))
                    rt = routp.tile([P, 512], DT, tag="rt")
                    nc.vector.tensor_scalar(rt[:], py[:], cmb_sb[:, tt:tt + 1],
                                            None, op0=ALU.mult)
                    routed_tiles[(j, dn)] = rt

            # shared gate/up
            hsT = hpool.tile([P, SC, 512], DTR, tag="hsT")
            for sc in range(SC):
                pg = ps_g.tile([P, 512], DT, tag="g")
                pu = ps_u.tile([P, 512], DT, tag="u")
                for dc in range(DC):
                    nc.tensor.matmul(pg[:], sg_sb[:, dc, sc * P:(sc + 1) * P],
                                     xT_sb[:, dc, tsl],
                                     start=(dc == 0), stop=(dc == DC - 1))
                for dc in range(DC):
                    nc.tensor.matmul(pu[:], su_sb[:, dc, sc * P:(sc + 1) * P],
                                     xT_sb[:, dc, tsl],
                                     start=(dc == 0), stop=(dc == DC - 1))
                sg_act = act.tile([P, 512], DT, tag="silu")
                nc.scalar.activation(sg_act[:], pg[:], AF.Silu)
                nc.vector.tensor_mul(hsT[:, sc], sg_act[:], pu[:])

            # shared down + add routed partial, write out
            for j in range(4):
                tt = tc_i * 4 + j
                for dn in range(2):
                    py = ps_y.tile([P, 512], DT, tag="y")
                    for sc in range(SC):
                        nc.tensor.matmul(py[:], hsT[:, sc, j * P:(j + 1) * P],
                                         sd_sb[:, sc, dn * 512:(dn + 1) * 512],
                                         start=(sc == 0), stop=(sc == SC - 1))
                    o_sb = outp.tile([P, 512], DT, tag="o")
                    nc.vector.tensor_add(o_sb[:], py[:], routed_tiles[(j, dn)][:])
                    nc.sync.dma_start(
                        out[tt * P:(tt + 1) * P, dn * 512:(dn + 1) * 512], o_sb[:])

    nc.compile()
    return nc


def _get_nc():
    global _NC_CACHE
    if _NC_CACHE is None:
        _NC_CACHE = _build_nc()
    return _NC_CACHE


def build_in_maps(inputs):
    x = np.ascontiguousarray(np.asarray(inputs["hidden_states"], dtype=np.float32))
    rw = np.ascontiguousarray(np.asarray(inputs["router_w"], dtype=np.float32))
    eg = np.asarray(inputs["experts_gate"], dtype=np.float32)
    eu = np.asarray(inputs["experts_up"], dtype=np.float32)
    ed = np.asarray(inputs["experts_down"], dtype=np.float32)
    sgf = np.asarray(inputs["shared_gate"], dtype=np.float32)
    suf = np.asarray(inputs["shared_up"], dtype=np.float32)
    sdf = np.asarray(inputs["shared_down"], dtype=np.float32)

    in_maps = []
    for c in range(NCORES):
        esel = np.zeros((P, E), dtype=np.float32)
        esel[:, c] = 1.0
        in_maps.append({
            "x": x,
            "rw": rw,
            "wg": np.ascontiguousarray(eg[c]),
            "wu": np.ascontiguousarray(eu[c]),
            "wd": np.ascontiguousarray(ed[c]),
            "sg": np.ascontiguousarray(sgf[:, c * FS:(c + 1) * FS]),
            "su": np.ascontiguousarray(suf[:, c * FS:(c + 1) * FS]),
            "sd": np.ascontiguousarray(sdf[c * FS:(c + 1) * FS, :]),
            "esel": esel,
        })
    return in_maps


def kernel(hidden_states, router_w, experts_gate, experts_up, experts_down,
           shared_gate, shared_up, shared_down):
    nc = _get_nc()
    in_maps = build_in_maps({
        "hidden_states": hidden_states, "router_w": router_w,
        "experts_gate": experts_gate, "experts_up": experts_up,
        "experts_down": experts_down, "shared_gate": shared_gate,
        "shared_up": shared_up, "shared_down": shared_down,
    })
    res = run_bass_kernel_spmd(nc, in_maps, core_ids=list(range(NCORES)))
    acc = res.results[0]["out"].astype(np.float32)
    for c in range(1, NCORES):
        acc = acc + res.results[c]["out"]
    return acc


# revision 5
# speedup vs baseline: 1.1770x; 1.1770x over previous
"""MoE layer (8 experts, top-2, shared expert) on 8 Trainium2 cores.

Sharding: expert-parallel. Core c holds expert c's gate/up/down weights and
a 1/8 tensor-parallel shard (256 cols) of the shared FFN. x and the router
are replicated; every core computes the full router (exact fp32) and its
expert's SwiGLU densely over all tokens, scales by its combine column, adds
its shared-FFN partial, and returns a [T, D] partial. Host sums the 8
partials — that sum is exactly routed + shared of the reference.

Expert/shared matmuls run in f32r (full PE rate, ~1.5e-4 rel err); the
router runs in true fp32 because the minimum top2-vs-top3 logit gap of the
workload (~3e-4) is too small for f32r noise.

Phases are interleaved per 512-token chunk (transpose+router, then expert
+shared FFN) so dense matmuls start early and keep the PE HAM clock warm;
bulk weight DMAs ride the gpsimd SWDGE queue so the x-tile loads on the
sync HWDGE queue aren't stuck behind them.
"""

import numpy as np
from contextlib import ExitStack

import concourse.bass as bass
import concourse.tile as tile
from concourse import bacc, mybir
from concourse.bass_utils import run_bass_kernel_spmd
from concourse.masks import make_identity

T, D, E = 2048, 1024, 8
F = 512          # per-expert FFN width
FS = 256         # shared FFN width per core (2048 / 8)
P = 128
NCORES = 8

TT = T // P      # 16 token tiles
DC = D // P      # 8 contraction chunks
FC = F // P      # 4 expert-f chunks
SC = FS // P     # 2 shared-f chunks
NTC = T // 512   # 4 token chunks of 512

DT = mybir.dt.float32
DTR = mybir.dt.float32r
AF = mybir.ActivationFunctionType
ALU = mybir.AluOpType
AX = mybir.AxisListType

_NC_CACHE = None


def _build_nc():
    nc = bacc.Bacc("TRN2", target_bir_lowering=False, debug=False,
                   num_devices=NCORES)
    x = nc.dram_tensor("x", [T, D], DT, kind="ExternalInput")
    rw = nc.dram_tensor("rw", [D, E], DT, kind="ExternalInput")
    wg = nc.dram_tensor("wg", [D, F], DT, kind="ExternalInput")
    wu = nc.dram_tensor("wu", [D, F], DT, kind="ExternalInput")
    wd = nc.dram_tensor("wd", [F, D], DT, kind="ExternalInput")
    sg = nc.dram_tensor("sg", [D, FS], DT, kind="ExternalInput")
    su = nc.dram_tensor("su", [D, FS], DT, kind="ExternalInput")
    sd = nc.dram_tensor("sd", [FS, D], DT, kind="ExternalInput")
    esel = nc.dram_tensor("esel", [P, E], DT, kind="ExternalInput")
    out = nc.dram_tensor("out", [T, D], DT, kind="ExternalOutput")

    with tile.TileContext(nc) as tc, ExitStack() as ctx:
        const = ctx.enter_context(tc.tile_pool(name="const", bufs=1))
        ident = const.tile([P, P], DT)
        make_identity(nc, ident[:])
        esel_sb = const.tile([P, E], DT)
        nc.sync.dma_start(esel_sb[:], esel[:])
        rw_sb = const.tile([P, DC, E], DT)
        nc.sync.dma_start(rw_sb[:], rw.rearrange("(c p) e -> p c e", p=P))

        # Bulk weights on the gpsimd SWDGE queue — keeps the sync HWDGE
        # queue free for the latency-critical x tiles.
        wgt = ctx.enter_context(tc.tile_pool(name="wgt", bufs=1))
        wg_sb = wgt.tile([P, DC, F], DTR)
        nc.gpsimd.dma_start(wg_sb[:], wg.rearrange("(c p) f -> p c f", p=P).bitcast(DTR))
        wu_sb = wgt.tile([P, DC, F], DTR)
        nc.gpsimd.dma_start(wu_sb[:], wu.rearrange("(c p) f -> p c f", p=P).bitcast(DTR))
        sg_sb = wgt.tile([P, DC, FS], DTR)
        nc.gpsimd.dma_start(sg_sb[:], sg.rearrange("(c p) f -> p c f", p=P).bitcast(DTR))
        su_sb = wgt.tile([P, DC, FS], DTR)
        nc.gpsimd.dma_start(su_sb[:], su.rearrange("(c p) f -> p c f", p=P).bitcast(DTR))
        wd_sb = wgt.tile([P, FC, D], DTR)
        nc.gpsimd.dma_start(wd_sb[:], wd.rearrange("(c p) d -> p c d", p=P).bitcast(DTR))
        sd_sb = wgt.tile([P, SC, D], DTR)
        nc.gpsimd.dma_start(sd_sb[:], sd.rearrange("(c p) d -> p c d", p=P).bitcast(DTR))

        big = ctx.enter_context(tc.tile_pool(name="big", bufs=1))
        xT_sb = big.tile([P, DC, T], DTR)   # transposed x, f32r-rounded
        cmb_sb = big.tile([P, TT], DT)      # combine weight column per token tile

        pha = ctx.enter_context(tc.tile_pool(name="pha", bufs=2))
        pha3 = ctx.enter_context(tc.tile_pool(name="pha3", bufs=3))
        rtr = ctx.enter_context(tc.tile_pool(name="rtr", bufs=2))
        act = ctx.enter_context(tc.tile_pool(name="act", bufs=2))
        hpool = ctx.enter_context(tc.tile_pool(name="hpool", bufs=1))
        routp = ctx.enter_context(tc.tile_pool(name="routp", bufs=8))
        outp = ctx.enter_context(tc.tile_pool(name="outp", bufs=3))

        # PSUM budget (8 banks): tr 2 + lg 1 + g 1 + u 1 + y 2 = 7
        ps_a = ctx.enter_context(tc.tile_pool(name="ps_a", bufs=2, space="PSUM"))
        ps_r = ctx.enter_context(tc.tile_pool(name="ps_r", bufs=1, space="PSUM"))
        ps_g = ctx.enter_context(tc.tile_pool(name="ps_g", bufs=1, space="PSUM"))
        ps_u = ctx.enter_context(tc.tile_pool(name="ps_u", bufs=1, space="PSUM"))
        ps_y = ctx.enter_context(tc.tile_pool(name="ps_y", bufs=2, space="PSUM"))

        def phase_a(tt):
            """Transpose one 128-token tile of x; exact-fp32 router for it."""
            x_sb = pha3.tile([P, D], DT, tag="x_in")
            nc.sync.dma_start(x_sb[:], x[tt * P:(tt + 1) * P, :])
            xrt = rtr.tile([P, DC, P], DT, tag="xrt")  # fp32 xT for the router
            for g in range(2):
                ps_tr = ps_a.tile([P, 4, P], DT, tag="tr")
                for j in range(4):
                    dc = g * 4 + j
                    nc.tensor.transpose(ps_tr[:, j], x_sb[:, dc * P:(dc + 1) * P],
                                        ident[:])
                # f32r copy feeds the expert/shared matmuls
                nc.scalar.copy(
                    xT_sb[:, g * 4:(g + 1) * 4, tt * P:(tt + 1) * P].bitcast(DTR),
                    ps_tr[:])
                # exact fp32 copy feeds the router
                nc.vector.tensor_copy(xrt[:, g * 4:(g + 1) * 4], ps_tr[:])

            ps_lg = ps_r.tile([P, E], DT, tag="lg")
            for dc in range(DC):
                nc.tensor.matmul(ps_lg[:], xrt[:, dc], rw_sb[:, dc],
                                 start=(dc == 0), stop=(dc == DC - 1))

            # top-2-of-8 softmax weights, renormalized; column e via esel
            m1 = pha.tile([P, 1], DT, tag="m1")
            nc.vector.reduce_max(out=m1[:], in_=ps_lg[:], axis=AX.X)
            nm1 = pha.tile([P, 1], DT, tag="nm1")
            nc.vector.tensor_scalar_mul(nm1[:], m1[:], -1.0)
            p_sb = pha.tile([P, E], DT, tag="p")
            nc.scalar.activation(p_sb[:], ps_lg[:], AF.Exp, bias=nm1[:])
            is1 = pha.tile([P, E], DT, tag="is1")
            nc.vector.tensor_scalar(is1[:], p_sb[:], 1.0, None, op0=ALU.is_ge)
            pm = pha.tile([P, E], DT, tag="pm")
            nc.vector.tensor_sub(pm[:], p_sb[:], is1[:])
            m2 = pha.tile([P, 1], DT, tag="m2")
            nc.vector.reduce_max(out=m2[:], in_=pm[:], axis=AX.X)
            s = pha.tile([P, 1], DT, tag="s")
            nc.vector.tensor_scalar_add(s[:], m2[:], 1.0)
            r = pha.tile([P, 1], DT, tag="r")
            nc.vector.reciprocal(r[:], s[:])
            t1 = pha.tile([P, E], DT, tag="t1")
            nc.vector.tensor_scalar(t1[:], p_sb[:], m2[:], r[:],
                                    op0=ALU.is_ge, op1=ALU.mult)
            w_sb = pha.tile([P, E], DT, tag="w")
            nc.vector.tensor_mul(w_sb[:], t1[:], p_sb[:])
            msk = pha.tile([P, E], DT, tag="msk")
            nc.vector.tensor_mul(msk[:], w_sb[:], esel_sb[:])
            nc.vector.reduce_sum(out=cmb_sb[:, tt:tt + 1], in_=msk[:], axis=AX.X)

        def phase_b(tc_i):
            """Expert SwiGLU + shared FFN shard for one 512-token chunk."""
            tsl = slice(tc_i * 512, (tc_i + 1) * 512)
            routed_tiles = {}
            hT = hpool.tile([P, FC, 512], DTR, tag="hT")
            for fc in range(FC):
                pg = ps_g.tile([P, 512], DT, tag="g")
                pu = ps_u.tile([P, 512], DT, tag="u")
                for dc in range(DC):
                    nc.tensor.matmul(pg[:], wg_sb[:, dc, fc * P:(fc + 1) * P],
                                     xT_sb[:, dc, tsl],
                                     start=(dc == 0), stop=(dc == DC - 1))
                for dc in range(DC):
                    nc.tensor.matmul(pu[:], wu_sb[:, dc, fc * P:(fc + 1) * P],
                                     xT_sb[:, dc, tsl],
                                     start=(dc == 0), stop=(dc == DC - 1))
                sg_act = act.tile([P, 512], DT, tag="silu")
                nc.scalar.activation(sg_act[:], pg[:], AF.Silu)
                nc.vector.tensor_mul(hT[:, fc], sg_act[:], pu[:])

            # expert down + combine-scale
            for j in range(4):
                tt = tc_i * 4 + j
                for dn in range(2):
                    py = ps_y.tile([P, 512], DT, tag="y")
                    for fc in range(FC):
                        nc.tensor.matmul(py[:], hT[:, fc, j * P:(j + 1) * P],
                                         wd_sb[:, fc, dn * 512:(dn + 1) * 512],
                                         start=(fc == 0), stop=(fc == FC - 1))
                    rt = routp.tile([P, 512], DT, tag="rt")
                    nc.vector.tensor_scalar(rt[:], py[:], cmb_sb[:, tt:tt + 1],
                                            None, op0=ALU.mult)
                    routed_tiles[(j, dn)] = rt

            # shared gate/up
            hsT = hpool.tile([P, SC, 512], DTR, tag="hsT")
            for sc in range(SC):
                pg = ps_g.tile([P, 512], DT, tag="g")
                pu = ps_u.tile([P, 512], DT, tag="u")
                for dc in range(DC):
                    nc.tensor.matmul(pg[:], sg_sb[:, dc, sc * P:(sc + 1) * P],
                                     xT_sb[:, dc, tsl],
                                     start=(dc == 0), stop=(dc == DC - 1))
                for dc in range(DC):
                    nc.tensor.matmul(pu[:], su_sb[:, dc, sc * P:(sc + 1) * P],
                                     xT_sb[:, dc, tsl],
                                     start=(dc == 0), stop=(dc == DC - 1))
                sg_act = act.tile([P, 512], DT, tag="silu")
                nc.scalar.activation(sg_act[:], pg[:], AF.Silu)
                nc.vector.tensor_mul(hsT[:, sc], sg_act[:], pu[:])

            # shared down + add routed partial, write out
            for j in range(4):
                tt = tc_i * 4 + j
                for dn in range(2):
                    py = ps_y.tile([P, 512], DT, tag="y")
                    for sc in range(SC):
                        nc.tensor.matmul(py[:], hsT[:, sc, j * P:(j + 1) * P],
                                         sd_sb[:, sc, dn * 512:(dn + 1) * 512],
                                         start=(sc == 0), stop=(sc == SC - 1))
                    o_sb = outp.tile([P, 512], DT, tag="o")
                    nc.vector.tensor_add(o_sb[:], py[:], routed_tiles[(j, dn)][:])
                    nc.sync.dma_start(
                        out[tt * P:(tt + 1) * P, dn * 512:(dn + 1) * 512], o_sb[:])

        for tc_i in range(NTC):
            for j in range(4):
                phase_a(tc_i * 4 + j)
            phase_b(tc_i)

    nc.compile()
    return nc


def _get_nc():
    global _NC_CACHE
    if _NC_CACHE is None:
        _NC_CACHE = _build_nc()
    return _NC_CACHE


def build_in_maps(inputs):
    x = np.ascontiguousarray(np.asarray(inputs["hidden_states"], dtype=np.float32))
    rw = np.ascontiguousarray(np.asarray(inputs["router_w"], dtype=np.float32))
    eg = np.asarray(inputs["experts_gate"], dtype=np.float32)
    eu = np.asarray(inputs["experts_up"], dtype=np.float32)
    ed = np.asarray(inputs["experts_down"], dtype=np.float32)
    sgf = np.asarray(inputs["shared_gate"], dtype=np.float32)
    suf = np.asarray(inputs["shared_up"], dtype=np.float32)
    sdf = np.asarray(inputs["shared_down"], dtype=np.float32)

    in_maps = []
    for c in range(NCORES):
        esel = np.zeros((P, E), dtype=np.float32)
        esel[:, c] = 1.0
        in_maps.append({
            "x": x,
            "rw": rw,
            "wg": np.ascontiguousarray(eg[c]),
            "wu": np.ascontiguousarray(eu[c]),
            "wd": np.ascontiguousarray(ed[c]),
            "sg": np.ascontiguousarray(sgf[:, c * FS:(c + 1) * FS]),
            "su": np.ascontiguousarray(suf[:, c * FS:(c + 1) * FS]),
            "sd": np.ascontiguousarray(sdf[c * FS:(c + 1) * FS, :]),
            "esel": esel,
        })
    return in_maps


def kernel(hidden_states, router_w, experts_gate, experts_up, experts_down,
           shared_gate, shared_up, shared_down):
    nc = _get_nc()
    in_maps = build_in_maps({
        "hidden_states": hidden_states, "router_w": router_w,
        "experts_gate": experts_gate, "experts_up": experts_up,
        "experts_down": experts_down, "shared_gate": shared_gate,
        "shared_up": shared_up, "shared_down": shared_down,
    })
    res = run_bass_kernel_spmd(nc, in_maps, core_ids=list(range(NCORES)))
    acc = res.results[0]["out"].astype(np.float32)
    for c in range(1, NCORES):
        acc = acc + res.results[c]["out"]
    return acc


# revision 6
# speedup vs baseline: 1.2192x; 1.0358x over previous
"""MoE layer (8 experts, top-2, shared expert) on 8 Trainium2 cores.

Sharding: expert-parallel. Core c holds expert c's gate/up/down weights and
a 1/8 tensor-parallel shard (256 cols) of the shared FFN. x and the router
are replicated; every core computes the full router (exact fp32) and its
expert's SwiGLU densely over all tokens, scales by its combine column, adds
its shared-FFN partial, and returns a [T, D] partial. Host sums the 8
partials — that sum is exactly routed + shared of the reference.

Expert/shared matmuls run in f32r (full PE rate, ~1.5e-4 rel err); the
router runs in true fp32 because the minimum top2-vs-top3 logit gap of the
workload (~3e-4) is too small for f32r noise.

Phases are interleaved per 512-token chunk (transpose+router, then expert
+shared FFN) so dense matmuls start early and keep the PE HAM clock warm;
bulk weight DMAs ride the gpsimd SWDGE queue so the x-tile loads on the
sync HWDGE queue aren't stuck behind them.
"""

import numpy as np
from contextlib import ExitStack

import concourse.bass as bass
import concourse.tile as tile
from concourse import bacc, mybir
from concourse.bass_utils import run_bass_kernel_spmd
from concourse.masks import make_identity

T, D, E = 2048, 1024, 8
F = 512          # per-expert FFN width
FS = 256         # shared FFN width per core (2048 / 8)
P = 128
NCORES = 8

TT = T // P      # 16 token tiles
DC = D // P      # 8 contraction chunks
FC = F // P      # 4 expert-f chunks
SC = FS // P     # 2 shared-f chunks
NTC = T // 512   # 4 token chunks of 512

DT = mybir.dt.float32
DTR = mybir.dt.float32r
AF = mybir.ActivationFunctionType
ALU = mybir.AluOpType
AX = mybir.AxisListType

_NC_CACHE = None


def _build_nc():
    nc = bacc.Bacc("TRN2", target_bir_lowering=False, debug=False,
                   num_devices=NCORES)
    x = nc.dram_tensor("x", [T, D], DT, kind="ExternalInput")
    rw = nc.dram_tensor("rw", [D, E], DT, kind="ExternalInput")
    wg = nc.dram_tensor("wg", [D, F], DT, kind="ExternalInput")
    wu = nc.dram_tensor("wu", [D, F], DT, kind="ExternalInput")
    wd = nc.dram_tensor("wd", [F, D], DT, kind="ExternalInput")
    sg = nc.dram_tensor("sg", [D, FS], DT, kind="ExternalInput")
    su = nc.dram_tensor("su", [D, FS], DT, kind="ExternalInput")
    sd = nc.dram_tensor("sd", [FS, D], DT, kind="ExternalInput")
    esel = nc.dram_tensor("esel", [P, E], DT, kind="ExternalInput")
    out = nc.dram_tensor("out", [T, D], DT, kind="ExternalOutput")

    with tile.TileContext(nc) as tc, ExitStack() as ctx:
        const = ctx.enter_context(tc.tile_pool(name="const", bufs=1))
        ident = const.tile([P, P], DT)
        make_identity(nc, ident[:])
        esel_sb = const.tile([P, E], DT)
        nc.sync.dma_start(esel_sb[:], esel[:])
        rw_sb = const.tile([P, DC, E], DT)
        nc.sync.dma_start(rw_sb[:], rw.rearrange("(c p) e -> p c e", p=P))

        # Bulk weights on the gpsimd SWDGE queue — keeps the sync HWDGE
        # queue free for the latency-critical x tiles.
        wgt = ctx.enter_context(tc.tile_pool(name="wgt", bufs=1))
        wg_sb = wgt.tile([P, DC, F], DTR)
        nc.scalar.dma_start(wg_sb[:], wg.rearrange("(c p) f -> p c f", p=P).bitcast(DTR))
        wu_sb = wgt.tile([P, DC, F], DTR)
        nc.scalar.dma_start(wu_sb[:], wu.rearrange("(c p) f -> p c f", p=P).bitcast(DTR))
        sg_sb = wgt.tile([P, DC, FS], DTR)
        nc.scalar.dma_start(sg_sb[:], sg.rearrange("(c p) f -> p c f", p=P).bitcast(DTR))
        su_sb = wgt.tile([P, DC, FS], DTR)
        nc.scalar.dma_start(su_sb[:], su.rearrange("(c p) f -> p c f", p=P).bitcast(DTR))
        wd_sb = wgt.tile([P, FC, D], DTR)
        nc.scalar.dma_start(wd_sb[:], wd.rearrange("(c p) d -> p c d", p=P).bitcast(DTR))
        sd_sb = wgt.tile([P, SC, D], DTR)
        nc.scalar.dma_start(sd_sb[:], sd.rearrange("(c p) d -> p c d", p=P).bitcast(DTR))

        big = ctx.enter_context(tc.tile_pool(name="big", bufs=1))
        xT_sb = big.tile([P, DC, T], DTR)   # transposed x, f32r-rounded
        cmb_sb = big.tile([P, TT], DT)      # combine weight column per token tile

        pha = ctx.enter_context(tc.tile_pool(name="pha", bufs=2))
        pha3 = ctx.enter_context(tc.tile_pool(name="pha3", bufs=3))
        rtr = ctx.enter_context(tc.tile_pool(name="rtr", bufs=2))
        act = ctx.enter_context(tc.tile_pool(name="act", bufs=2))
        hpool = ctx.enter_context(tc.tile_pool(name="hpool", bufs=1))
        routp = ctx.enter_context(tc.tile_pool(name="routp", bufs=8))
        outp = ctx.enter_context(tc.tile_pool(name="outp", bufs=3))

        # PSUM budget (8 banks): tr 2 + lg 1 + g 1 + u 1 + y 2 = 7
        ps_a = ctx.enter_context(tc.tile_pool(name="ps_a", bufs=2, space="PSUM"))
        ps_r = ctx.enter_context(tc.tile_pool(name="ps_r", bufs=1, space="PSUM"))
        ps_g = ctx.enter_context(tc.tile_pool(name="ps_g", bufs=1, space="PSUM"))
        ps_u = ctx.enter_context(tc.tile_pool(name="ps_u", bufs=1, space="PSUM"))
        ps_y = ctx.enter_context(tc.tile_pool(name="ps_y", bufs=2, space="PSUM"))

        def phase_a(tt):
            """Transpose one 128-token tile of x; exact-fp32 router for it."""
            x_sb = pha3.tile([P, D], DT, tag="x_in")
            nc.sync.dma_start(x_sb[:], x[tt * P:(tt + 1) * P, :])
            xrt = rtr.tile([P, DC, P], DT, tag="xrt")  # fp32 xT for the router
            for g in range(2):
                ps_tr = ps_a.tile([P, 4, P], DT, tag="tr")
                for j in range(4):
                    dc = g * 4 + j
                    nc.tensor.transpose(ps_tr[:, j], x_sb[:, dc * P:(dc + 1) * P],
                                        ident[:])
                # f32r copy feeds the expert/shared matmuls
                nc.scalar.copy(
                    xT_sb[:, g * 4:(g + 1) * 4, tt * P:(tt + 1) * P].bitcast(DTR),
                    ps_tr[:])
                # exact fp32 copy feeds the router
                nc.vector.tensor_copy(xrt[:, g * 4:(g + 1) * 4], ps_tr[:])

            ps_lg = ps_r.tile([P, E], DT, tag="lg")
            for dc in range(DC):
                nc.tensor.matmul(ps_lg[:], xrt[:, dc], rw_sb[:, dc],
                                 start=(dc == 0), stop=(dc == DC - 1))

            # top-2-of-8 softmax weights, renormalized; column e via esel
            m1 = pha.tile([P, 1], DT, tag="m1")
            nc.vector.reduce_max(out=m1[:], in_=ps_lg[:], axis=AX.X)
            nm1 = pha.tile([P, 1], DT, tag="nm1")
            nc.vector.tensor_scalar_mul(nm1[:], m1[:], -1.0)
            p_sb = pha.tile([P, E], DT, tag="p")
            nc.scalar.activation(p_sb[:], ps_lg[:], AF.Exp, bias=nm1[:])
            is1 = pha.tile([P, E], DT, tag="is1")
            nc.vector.tensor_scalar(is1[:], p_sb[:], 1.0, None, op0=ALU.is_ge)
            pm = pha.tile([P, E], DT, tag="pm")
            nc.vector.tensor_sub(pm[:], p_sb[:], is1[:])
            m2 = pha.tile([P, 1], DT, tag="m2")
            nc.vector.reduce_max(out=m2[:], in_=pm[:], axis=AX.X)
            s = pha.tile([P, 1], DT, tag="s")
            nc.vector.tensor_scalar_add(s[:], m2[:], 1.0)
            r = pha.tile([P, 1], DT, tag="r")
            nc.vector.reciprocal(r[:], s[:])
            t1 = pha.tile([P, E], DT, tag="t1")
            nc.vector.tensor_scalar(t1[:], p_sb[:], m2[:], r[:],
                                    op0=ALU.is_ge, op1=ALU.mult)
            w_sb = pha.tile([P, E], DT, tag="w")
            nc.vector.tensor_mul(w_sb[:], t1[:], p_sb[:])
            msk = pha.tile([P, E], DT, tag="msk")
            nc.vector.tensor_mul(msk[:], w_sb[:], esel_sb[:])
            nc.vector.reduce_sum(out=cmb_sb[:, tt:tt + 1], in_=msk[:], axis=AX.X)

        def phase_b(tc_i, next_tts):
            """Expert SwiGLU + shared FFN shard for one 512-token chunk.
            Interleaves the next chunk's transposes/router between dense
            blocks so the PE never sees a long dense-matmul-free stretch."""
            tsl = slice(tc_i * 512, (tc_i + 1) * 512)
            routed_tiles = {}
            hT = hpool.tile([P, FC, 512], DTR, tag="hT")
            for fc in range(FC):
                if fc > 0 and len(next_tts) >= fc:
                    phase_a(next_tts[fc - 1])
                pg = ps_g.tile([P, 512], DT, tag="g")
                pu = ps_u.tile([P, 512], DT, tag="u")
                for dc in range(DC):
                    nc.tensor.matmul(pg[:], wg_sb[:, dc, fc * P:(fc + 1) * P],
                                     xT_sb[:, dc, tsl],
                                     start=(dc == 0), stop=(dc == DC - 1))
                for dc in range(DC):
                    nc.tensor.matmul(pu[:], wu_sb[:, dc, fc * P:(fc + 1) * P],
                                     xT_sb[:, dc, tsl],
                                     start=(dc == 0), stop=(dc == DC - 1))
                sg_act = act.tile([P, 512], DT, tag="silu")
                nc.scalar.activation(sg_act[:], pg[:], AF.Silu)
                nc.vector.tensor_mul(hT[:, fc], sg_act[:], pu[:])

            if len(next_tts) == 4:
                phase_a(next_tts[3])

            # expert down + combine-scale
            for j in range(4):
                tt = tc_i * 4 + j
                for dn in range(2):
                    py = ps_y.tile([P, 512], DT, tag="y")
                    for fc in range(FC):
                        nc.tensor.matmul(py[:], hT[:, fc, j * P:(j + 1) * P],
                                         wd_sb[:, fc, dn * 512:(dn + 1) * 512],
                                         start=(fc == 0), stop=(fc == FC - 1))
                    rt = routp.tile([P, 512], DT, tag="rt")
                    nc.vector.tensor_scalar(rt[:], py[:], cmb_sb[:, tt:tt + 1],
                                            None, op0=ALU.mult)
                    routed_tiles[(j, dn)] = rt

            # shared gate/up
            hsT = hpool.tile([P, SC, 512], DTR, tag="hsT")
            for sc in range(SC):
                pg = ps_g.tile([P, 512], DT, tag="g")
                pu = ps_u.tile([P, 512], DT, tag="u")
                for dc in range(DC):
                    nc.tensor.matmul(pg[:], sg_sb[:, dc, sc * P:(sc + 1) * P],
                                     xT_sb[:, dc, tsl],
                                     start=(dc == 0), stop=(dc == DC - 1))
                for dc in range(DC):
                    nc.tensor.matmul(pu[:], su_sb[:, dc, sc * P:(sc + 1) * P],
                                     xT_sb[:, dc, tsl],
                                     start=(dc == 0), stop=(dc == DC - 1))
                sg_act = act.tile([P, 512], DT, tag="silu")
                nc.scalar.activation(sg_act[:], pg[:], AF.Silu)
                nc.vector.tensor_mul(hsT[:, sc], sg_act[:], pu[:])

            # shared down + add routed partial, write out
            for j in range(4):
                tt = tc_i * 4 + j
                for dn in range(2):
                    py = ps_y.tile([P, 512], DT, tag="y")
                    for sc in range(SC):
                        nc.tensor.matmul(py[:], hsT[:, sc, j * P:(j + 1) * P],
                                         sd_sb[:, sc, dn * 512:(dn + 1) * 512],
                                         start=(sc == 0), stop=(sc == SC - 1))
                    o_sb = outp.tile([P, 512], DT, tag="o")
                    nc.vector.tensor_add(o_sb[:], py[:], routed_tiles[(j, dn)][:])
                    nc.sync.dma_start(
                        out[tt * P:(tt + 1) * P, dn * 512:(dn + 1) * 512], o_sb[:])

        for j in range(4):
            phase_a(j)
        for tc_i in range(NTC):
            nxt = [tc_i * 4 + 4 + j for j in range(4)] if tc_i + 1 < NTC else []
            phase_b(tc_i, nxt)

    nc.compile()
    return nc


def _get_nc():
    global _NC_CACHE
    if _NC_CACHE is None:
        _NC_CACHE = _build_nc()
    return _NC_CACHE


def build_in_maps(inputs):
    x = np.ascontiguousarray(np.asarray(inputs["hidden_states"], dtype=np.float32))
    rw = np.ascontiguousarray(np.asarray(inputs["router_w"], dtype=np.float32))
    eg = np.asarray(inputs["experts_gate"], dtype=np.float32)
    eu = np.asarray(inputs["experts_up"], dtype=np.float32)
    ed = np.asarray(inputs["experts_down"], dtype=np.float32)
    sgf = np.asarray(inputs["shared_gate"], dtype=np.float32)
    suf = np.asarray(inputs["shared_up"], dtype=np.float32)
    sdf = np.asarray(inputs["shared_down"], dtype=np.float32)

    in_maps = []
    for c in range(NCORES):
        esel = np.zeros((P, E), dtype=np.float32)
        esel[:, c] = 1.0
        in_maps.append({
            "x": x,
            "rw": rw,
            "wg": np.ascontiguousarray(eg[c]),
            "wu": np.ascontiguousarray(eu[c]),
            "wd": np.ascontiguousarray(ed[c]),
            "sg": np.ascontiguousarray(sgf[:, c * FS:(c + 1) * FS]),
            "su": np.ascontiguousarray(suf[:, c * FS:(c + 1) * FS]),
            "sd": np.ascontiguousarray(sdf[c * FS:(c + 1) * FS, :]),
            "esel": esel,
        })
    return in_maps


def kernel(hidden_states, router_w, experts_gate, experts_up, experts_down,
           shared_gate, shared_up, shared_down):
    nc = _get_nc()
    in_maps = build_in_maps({
        "hidden_states": hidden_states, "router_w": router_w,
        "experts_gate": experts_gate, "experts_up": experts_up,
        "experts_down": experts_down, "shared_gate": shared_gate,
        "shared_up": shared_up, "shared_down": shared_down,
    })
    res = run_bass_kernel_spmd(nc, in_maps, core_ids=list(range(NCORES)))
    acc = res.results[0]["out"].astype(np.float32)
    for c in range(1, NCORES):
        acc = acc + res.results[c]["out"]
    return acc


# revision 7
# speedup vs baseline: 1.2289x; 1.0080x over previous
"""MoE layer (8 experts, top-2, shared expert) on 8 Trainium2 cores.

Sharding: expert-parallel. Core c holds expert c's gate/up/down weights and
a 1/8 tensor-parallel shard (256 cols) of the shared FFN. x (replicated,
passed transposed as part of the host-side sharding relayout) and the
router are on every core; every core computes the full router (exact fp32)
and its expert's SwiGLU densely over all tokens, scales by its combine
column, adds its shared-FFN partial, and returns a [T, D] partial. The
host-side unshard sums the 8 partials — exactly routed + shared of the
reference.

Expert/shared matmuls run in f32r (full PE rate at moving-dim >= 256,
~1.5e-4 rel err). The router runs in true fp32: the SBUF copy of xT is
DMA'd (bit-exact) into an f32r-typed tile, which the dense matmuls consume
as f32r while the router matmuls read the same bits bitcast back to fp32 —
the workload's minimum top2-vs-top3 logit gap (~3e-4) is too small for
f32r noise but is 300x the fp32 matmul error.
"""

import numpy as np
from contextlib import ExitStack

import concourse.bass as bass
import concourse.tile as tile
from concourse import bacc, mybir
from concourse.bass_utils import run_bass_kernel_spmd

T, D, E = 2048, 1024, 8
F = 512          # per-expert FFN width
FS = 256         # shared FFN width per core (2048 / 8)
P = 128
NCORES = 8

TT = T // P      # 16 token tiles
DC = D // P      # 8 contraction chunks
FC = F // P      # 4 expert-f chunks
SC = FS // P     # 2 shared-f chunks
NTC = T // 512   # 4 token chunks of 512

DT = mybir.dt.float32
DTR = mybir.dt.float32r
AF = mybir.ActivationFunctionType
ALU = mybir.AluOpType
AX = mybir.AxisListType

_NC_CACHE = None


def _build_nc():
    nc = bacc.Bacc("TRN2", target_bir_lowering=False, debug=False,
                   num_devices=NCORES)
    xt = nc.dram_tensor("xt", [D, T], DT, kind="ExternalInput")
    rw = nc.dram_tensor("rw", [D, E], DT, kind="ExternalInput")
    wg = nc.dram_tensor("wg", [D, F], DT, kind="ExternalInput")
    wu = nc.dram_tensor("wu", [D, F], DT, kind="ExternalInput")
    wd = nc.dram_tensor("wd", [F, D], DT, kind="ExternalInput")
    sg = nc.dram_tensor("sg", [D, FS], DT, kind="ExternalInput")
    su = nc.dram_tensor("su", [D, FS], DT, kind="ExternalInput")
    sd = nc.dram_tensor("sd", [FS, D], DT, kind="ExternalInput")
    esel = nc.dram_tensor("esel", [P, E], DT, kind="ExternalInput")
    out = nc.dram_tensor("out", [T, D], DT, kind="ExternalOutput")

    xtr = xt.rearrange("(c p) t -> p c t", p=P)  # [P, DC, T] view

    with tile.TileContext(nc) as tc, ExitStack() as ctx:
        const = ctx.enter_context(tc.tile_pool(name="const", bufs=1))
        esel_sb = const.tile([P, E], DT)
        nc.sync.dma_start(esel_sb[:], esel[:])
        rw_sb = const.tile([P, DC, E], DT)
        nc.sync.dma_start(rw_sb[:], rw.rearrange("(c p) e -> p c e", p=P))

        big = ctx.enter_context(tc.tile_pool(name="big", bufs=1))
        xT_sb = big.tile([P, DC, T], DTR)   # transposed x (bit-exact fp32)
        cmb_sb = big.tile([P, TT], DT)      # combine weight column per token tile

        # xT: first chunk split in half for latency-to-first-matmul, then
        # one DMA per 512-token chunk (2KB descriptors).
        nc.sync.dma_start(xT_sb[:, :, 0:256], xtr[:, :, 0:256].bitcast(DTR))
        nc.sync.dma_start(xT_sb[:, :, 256:512], xtr[:, :, 256:512].bitcast(DTR))
        for tc_i in range(1, NTC):
            tsl = slice(tc_i * 512, (tc_i + 1) * 512)
            nc.sync.dma_start(xT_sb[:, :, tsl], xtr[:, :, tsl].bitcast(DTR))

        # Bulk weights on the scalar-engine HWDGE ring — keeps the sync ring
        # free for xT and the output stores.
        wgt = ctx.enter_context(tc.tile_pool(name="wgt", bufs=1))
        wg_sb = wgt.tile([P, DC, F], DTR)
        nc.scalar.dma_start(wg_sb[:], wg.rearrange("(c p) f -> p c f", p=P).bitcast(DTR))
        wu_sb = wgt.tile([P, DC, F], DTR)
        nc.scalar.dma_start(wu_sb[:], wu.rearrange("(c p) f -> p c f", p=P).bitcast(DTR))
        sg_sb = wgt.tile([P, DC, FS], DTR)
        nc.scalar.dma_start(sg_sb[:], sg.rearrange("(c p) f -> p c f", p=P).bitcast(DTR))
        su_sb = wgt.tile([P, DC, FS], DTR)
        nc.scalar.dma_start(su_sb[:], su.rearrange("(c p) f -> p c f", p=P).bitcast(DTR))
        wd_sb = wgt.tile([P, FC, D], DTR)
        nc.scalar.dma_start(wd_sb[:], wd.rearrange("(c p) d -> p c d", p=P).bitcast(DTR))
        sd_sb = wgt.tile([P, SC, D], DTR)
        nc.scalar.dma_start(sd_sb[:], sd.rearrange("(c p) d -> p c d", p=P).bitcast(DTR))

        pha = ctx.enter_context(tc.tile_pool(name="pha", bufs=2))
        act = ctx.enter_context(tc.tile_pool(name="act", bufs=2))
        hpool = ctx.enter_context(tc.tile_pool(name="hpool", bufs=2))
        routp = ctx.enter_context(tc.tile_pool(name="routp", bufs=8))
        outp = ctx.enter_context(tc.tile_pool(name="outp", bufs=3))

        # PSUM budget (8 banks): lg 2 + g 2 + u 2 + y 2 = 8
        ps_r = ctx.enter_context(tc.tile_pool(name="ps_r", bufs=2, space="PSUM"))
        ps_g = ctx.enter_context(tc.tile_pool(name="ps_g", bufs=2, space="PSUM"))
        ps_u = ctx.enter_context(tc.tile_pool(name="ps_u", bufs=2, space="PSUM"))
        ps_y = ctx.enter_context(tc.tile_pool(name="ps_y", bufs=2, space="PSUM"))

        def router(tt):
            """Exact-fp32 router + top-2 combine weights for one token tile."""
            ps_lg = ps_r.tile([P, E], DT, tag="lg")
            csl = slice(tt * P, (tt + 1) * P)
            for dc in range(DC):
                nc.tensor.matmul(ps_lg[:], xT_sb[:, dc, csl].bitcast(DT),
                                 rw_sb[:, dc],
                                 start=(dc == 0), stop=(dc == DC - 1))

            m1 = pha.tile([P, 1], DT, tag="m1")
            nc.vector.reduce_max(out=m1[:], in_=ps_lg[:], axis=AX.X)
            nm1 = pha.tile([P, 1], DT, tag="nm1")
            nc.vector.tensor_scalar_mul(nm1[:], m1[:], -1.0)
            p_sb = pha.tile([P, E], DT, tag="p")
            nc.scalar.activation(p_sb[:], ps_lg[:], AF.Exp, bias=nm1[:])
            is1 = pha.tile([P, E], DT, tag="is1")
            nc.vector.tensor_scalar(is1[:], p_sb[:], 1.0, None, op0=ALU.is_ge)
            pm = pha.tile([P, E], DT, tag="pm")
            nc.vector.tensor_sub(pm[:], p_sb[:], is1[:])
            m2 = pha.tile([P, 1], DT, tag="m2")
            nc.vector.reduce_max(out=m2[:], in_=pm[:], axis=AX.X)
            s = pha.tile([P, 1], DT, tag="s")
            nc.vector.tensor_scalar_add(s[:], m2[:], 1.0)
            r = pha.tile([P, 1], DT, tag="r")
            nc.vector.reciprocal(r[:], s[:])
            t1 = pha.tile([P, E], DT, tag="t1")
            nc.vector.tensor_scalar(t1[:], p_sb[:], m2[:], r[:],
                                    op0=ALU.is_ge, op1=ALU.mult)
            w_sb = pha.tile([P, E], DT, tag="w")
            nc.vector.tensor_mul(w_sb[:], t1[:], p_sb[:])
            msk = pha.tile([P, E], DT, tag="msk")
            nc.vector.tensor_mul(msk[:], w_sb[:], esel_sb[:])
            nc.vector.reduce_sum(out=cmb_sb[:, tt:tt + 1], in_=msk[:], axis=AX.X)

        def phase_b(tc_i):
            """Expert SwiGLU + shared FFN shard for one 512-token chunk."""
            tsl = slice(tc_i * 512, (tc_i + 1) * 512)
            routed_tiles = {}
            hT = hpool.tile([P, FC, 512], DTR, tag="hT")
            for fc in range(FC):
                pg = ps_g.tile([P, 512], DT, tag="g")
                pu = ps_u.tile([P, 512], DT, tag="u")
                for dc in range(DC):
                    nc.tensor.matmul(pg[:], wg_sb[:, dc, fc * P:(fc + 1) * P],
                                     xT_sb[:, dc, tsl],
                                     start=(dc == 0), stop=(dc == DC - 1))
                for dc in range(DC):
                    nc.tensor.matmul(pu[:], wu_sb[:, dc, fc * P:(fc + 1) * P],
                                     xT_sb[:, dc, tsl],
                                     start=(dc == 0), stop=(dc == DC - 1))
                sg_act = act.tile([P, 512], DT, tag="silu")
                nc.scalar.activation(sg_act[:], pg[:], AF.Silu)
                nc.vector.tensor_mul(hT[:, fc], sg_act[:], pu[:])

            # expert down + combine-scale
            for j in range(4):
                tt = tc_i * 4 + j
                for dn in range(2):
                    py = ps_y.tile([P, 512], DT, tag="y")
                    for fc in range(FC):
                        nc.tensor.matmul(py[:], hT[:, fc, j * P:(j + 1) * P],
                                         wd_sb[:, fc, dn * 512:(dn + 1) * 512],
                                         start=(fc == 0), stop=(fc == FC - 1))
                    rt = routp.tile([P, 512], DT, tag="rt")
                    nc.vector.tensor_scalar(rt[:], py[:], cmb_sb[:, tt:tt + 1],
                                            None, op0=ALU.mult)
                    routed_tiles[(j, dn)] = rt

            # shared gate/up
            hsT = hpool.tile([P, SC, 512], DTR, tag="hsT")
            for sc in range(SC):
                pg = ps_g.tile([P, 512], DT, tag="g")
                pu = ps_u.tile([P, 512], DT, tag="u")
                for dc in range(DC):
                    nc.tensor.matmul(pg[:], sg_sb[:, dc, sc * P:(sc + 1) * P],
                                     xT_sb[:, dc, tsl],
                                     start=(dc == 0), stop=(dc == DC - 1))
                for dc in range(DC):
                    nc.tensor.matmul(pu[:], su_sb[:, dc, sc * P:(sc + 1) * P],
                                     xT_sb[:, dc, tsl],
                                     start=(dc == 0), stop=(dc == DC - 1))
                sg_act = act.tile([P, 512], DT, tag="silu")
                nc.scalar.activation(sg_act[:], pg[:], AF.Silu)
                nc.vector.tensor_mul(hsT[:, sc], sg_act[:], pu[:])

            # shared down + add routed partial, write out
            for j in range(4):
                tt = tc_i * 4 + j
                for dn in range(2):
                    py = ps_y.tile([P, 512], DT, tag="y")
                    for sc in range(SC):
                        nc.tensor.matmul(py[:], hsT[:, sc, j * P:(j + 1) * P],
                                         sd_sb[:, sc, dn * 512:(dn + 1) * 512],
                                         start=(sc == 0), stop=(sc == SC - 1))
                    o_sb = outp.tile([P, 512], DT, tag="o")
                    nc.vector.tensor_add(o_sb[:], py[:], routed_tiles[(j, dn)][:])
                    nc.sync.dma_start(
                        out[tt * P:(tt + 1) * P, dn * 512:(dn + 1) * 512], o_sb[:])

        for tc_i in range(NTC):
            for j in range(4):
                router(tc_i * 4 + j)
            phase_b(tc_i)

    nc.compile()
    return nc


def _get_nc():
    global _NC_CACHE
    if _NC_CACHE is None:
        _NC_CACHE = _build_nc()
    return _NC_CACHE


def build_in_maps(inputs):
    x = np.asarray(inputs["hidden_states"], dtype=np.float32)
    xt = np.ascontiguousarray(x.T)
    rw = np.ascontiguousarray(np.asarray(inputs["router_w"], dtype=np.float32))
    eg = np.asarray(inputs["experts_gate"], dtype=np.float32)
    eu = np.asarray(inputs["experts_up"], dtype=np.float32)
    ed = np.asarray(inputs["experts_down"], dtype=np.float32)
    sgf = np.asarray(inputs["shared_gate"], dtype=np.float32)
    suf = np.asarray(inputs["shared_up"], dtype=np.float32)
    sdf = np.asarray(inputs["shared_down"], dtype=np.float32)

    in_maps = []
    for c in range(NCORES):
        esel = np.zeros((P, E), dtype=np.float32)
        esel[:, c] = 1.0
        in_maps.append({
            "xt": xt,
            "rw": rw,
            "wg": np.ascontiguousarray(eg[c]),
            "wu": np.ascontiguousarray(eu[c]),
            "wd": np.ascontiguousarray(ed[c]),
            "sg": np.ascontiguousarray(sgf[:, c * FS:(c + 1) * FS]),
            "su": np.ascontiguousarray(suf[:, c * FS:(c + 1) * FS]),
            "sd": np.ascontiguousarray(sdf[c * FS:(c + 1) * FS, :]),
            "esel": esel,
        })
    return in_maps


def kernel(hidden_states, router_w, experts_gate, experts_up, experts_down,
           shared_gate, shared_up, shared_down):
    nc = _get_nc()
    in_maps = build_in_maps({
        "hidden_states": hidden_states, "router_w": router_w,
        "experts_gate": experts_gate, "experts_up": experts_up,
        "experts_down": experts_down, "shared_gate": shared_gate,
        "shared_up": shared_up, "shared_down": shared_down,
    })
    res = run_bass_kernel_spmd(nc, in_maps, core_ids=list(range(NCORES)))
    acc = res.results[0]["out"].astype(np.float32)
    for c in range(1, NCORES):
        acc = acc + res.results[c]["out"]
    return acc


# revision 9
# speedup vs baseline: 1.2973x; 1.0556x over previous
"""MoE layer (8 experts, top-2, shared expert) on 8 Trainium2 cores.

Sharding: expert-parallel. Core c holds expert c's gate/up/down weights and
a 1/8 tensor-parallel shard (256 cols) of the shared FFN. x and the router
are replicated; every core computes the full router (exact fp32) and its
expert's SwiGLU densely over all tokens, scales by its combine column, adds
its shared-FFN partial, and returns a [T, D] partial. The host-side unshard
sums the 8 partials — exactly routed + shared of the reference.

All inputs are relaid out host-side during sharding so that every DMA is
contiguous per SBUF partition (128 big descriptors per transfer instead of
1024 small ones — HWDGE descriptor generation was the startup bottleneck).

Expert/shared matmuls run in f32r (full PE rate at moving-dim >= 256,
~1.5e-4 rel err). The router runs in true fp32: xT is DMA'd bit-exact into
an f32r-typed tile; dense matmuls consume it as f32r (the PE rounds
internally) while router matmuls read the same bits bitcast back to fp32.
The workload's minimum top2-vs-top3 logit gap (~3e-4) is too small for
f32r noise but is ~300x the fp32 matmul error.
"""

import numpy as np
from contextlib import ExitStack

import concourse.bass as bass
import concourse.tile as tile
from concourse import bacc, mybir
from concourse.bass_utils import run_bass_kernel_spmd

T, D, E = 2048, 1024, 8
F = 512          # per-expert FFN width
FS = 256         # shared FFN width per core (2048 / 8)
P = 128
NCORES = 8

TT = T // P      # 16 token tiles
DC = D // P      # 8 contraction chunks
FC = F // P      # 4 expert-f chunks
SC = FS // P     # 2 shared-f chunks
NTC = T // 512   # 4 token chunks of 512

DT = mybir.dt.float32
DTR = mybir.dt.float32r
AF = mybir.ActivationFunctionType
ALU = mybir.AluOpType
AX = mybir.AxisListType

_NC_CACHE = None


def _build_nc():
    nc = bacc.Bacc("TRN2", target_bir_lowering=False, debug=False,
                   num_devices=NCORES)
    # all inputs pre-relaid out host-side for partition-contiguous DMA
    xt = nc.dram_tensor("xt", [NTC, P, DC, 512], DT, kind="ExternalInput")
    rw = nc.dram_tensor("rw", [P, DC, E], DT, kind="ExternalInput")
    wg = nc.dram_tensor("wg", [P, DC, F], DT, kind="ExternalInput")
    wu = nc.dram_tensor("wu", [P, DC, F], DT, kind="ExternalInput")
    wd = nc.dram_tensor("wd", [P, FC, D], DT, kind="ExternalInput")
    sg = nc.dram_tensor("sg", [P, DC, FS], DT, kind="ExternalInput")
    su = nc.dram_tensor("su", [P, DC, FS], DT, kind="ExternalInput")
    sd = nc.dram_tensor("sd", [P, SC, D], DT, kind="ExternalInput")
    esel = nc.dram_tensor("esel", [P, E], DT, kind="ExternalInput")
    out = nc.dram_tensor("out", [P, TT, D], DT, kind="ExternalOutput")

    with tile.TileContext(nc) as tc, ExitStack() as ctx:
        const = ctx.enter_context(tc.tile_pool(name="const", bufs=1))
        esel_sb = const.tile([P, E], DT)
        nc.sync.dma_start(esel_sb[:], esel[:])
        rw_sb = const.tile([P, DC, E], DT)
        nc.sync.dma_start(rw_sb[:], rw[:])

        big = ctx.enter_context(tc.tile_pool(name="big", bufs=1))
        xT_sb = big.tile([P, NTC, DC, 512], DTR)  # transposed x (bit-exact fp32)
        cmb_sb = big.tile([P, TT], DT)            # combine column per token tile

        for tc_i in range(NTC):
            nc.sync.dma_start(xT_sb[:, tc_i], xt[tc_i].bitcast(DTR))

        # Bulk weights on the scalar-engine HWDGE ring — keeps the sync ring
        # free for xT and the output stores.
        wgt = ctx.enter_context(tc.tile_pool(name="wgt", bufs=1))
        wg_sb = wgt.tile([P, DC, F], DTR)
        nc.scalar.dma_start(wg_sb[:], wg[:].bitcast(DTR))
        wu_sb = wgt.tile([P, DC, F], DTR)
        nc.scalar.dma_start(wu_sb[:], wu[:].bitcast(DTR))
        sg_sb = wgt.tile([P, DC, FS], DTR)
        nc.scalar.dma_start(sg_sb[:], sg[:].bitcast(DTR))
        su_sb = wgt.tile([P, DC, FS], DTR)
        nc.scalar.dma_start(su_sb[:], su[:].bitcast(DTR))
        wd_sb = wgt.tile([P, FC, D], DTR)
        nc.scalar.dma_start(wd_sb[:], wd[:].bitcast(DTR))
        sd_sb = wgt.tile([P, SC, D], DTR)
        nc.scalar.dma_start(sd_sb[:], sd[:].bitcast(DTR))

        pha = ctx.enter_context(tc.tile_pool(name="pha", bufs=2))
        act = ctx.enter_context(tc.tile_pool(name="act", bufs=2))
        hpool = ctx.enter_context(tc.tile_pool(name="hpool", bufs=2))
        outp = ctx.enter_context(tc.tile_pool(name="outp", bufs=3))

        # PSUM budget (8 banks): lg 2 + g 2 + u 2 + y1 1 + y2 1 = 8
        ps_r = ctx.enter_context(tc.tile_pool(name="ps_r", bufs=2, space="PSUM"))
        ps_g = ctx.enter_context(tc.tile_pool(name="ps_g", bufs=2, space="PSUM"))
        ps_u = ctx.enter_context(tc.tile_pool(name="ps_u", bufs=2, space="PSUM"))
        ps_y1 = ctx.enter_context(tc.tile_pool(name="ps_y1", bufs=1, space="PSUM"))
        ps_y2 = ctx.enter_context(tc.tile_pool(name="ps_y2", bufs=1, space="PSUM"))

        def router(tt):
            """Exact-fp32 router + top-2 combine weights for one token tile."""
            tci, j = divmod(tt, 4)
            csl = slice(j * P, (j + 1) * P)
            ps_lg = ps_r.tile([P, E], DT, tag="lg")
            for dc in range(DC):
                nc.tensor.matmul(ps_lg[:], xT_sb[:, tci, dc, csl].bitcast(DT),
                                 rw_sb[:, dc],
                                 start=(dc == 0), stop=(dc == DC - 1))

            m1 = pha.tile([P, 1], DT, tag="m1")
            nc.vector.reduce_max(out=m1[:], in_=ps_lg[:], axis=AX.X)
            nm1 = pha.tile([P, 1], DT, tag="nm1")
            nc.vector.tensor_scalar_mul(nm1[:], m1[:], -1.0)
            p_sb = pha.tile([P, E], DT, tag="p")
            nc.scalar.activation(p_sb[:], ps_lg[:], AF.Exp, bias=nm1[:])
            is1 = pha.tile([P, E], DT, tag="is1")
            nc.vector.tensor_scalar(is1[:], p_sb[:], 1.0, None, op0=ALU.is_ge)
            pm = pha.tile([P, E], DT, tag="pm")
            nc.vector.tensor_sub(pm[:], p_sb[:], is1[:])
            m2 = pha.tile([P, 1], DT, tag="m2")
            nc.vector.reduce_max(out=m2[:], in_=pm[:], axis=AX.X)
            s = pha.tile([P, 1], DT, tag="s")
            nc.vector.tensor_scalar_add(s[:], m2[:], 1.0)
            r = pha.tile([P, 1], DT, tag="r")
            nc.vector.reciprocal(r[:], s[:])
            t1 = pha.tile([P, E], DT, tag="t1")
            nc.vector.tensor_scalar(t1[:], p_sb[:], m2[:], r[:],
                                    op0=ALU.is_ge, op1=ALU.mult)
            w_sb = pha.tile([P, E], DT, tag="w")
            nc.vector.tensor_mul(w_sb[:], t1[:], p_sb[:])
            msk = pha.tile([P, E], DT, tag="msk")
            nc.vector.tensor_mul(msk[:], w_sb[:], esel_sb[:])
            nc.vector.reduce_sum(out=cmb_sb[:, tt:tt + 1], in_=msk[:], axis=AX.X)

        def phase_b(tc_i):
            """Expert SwiGLU + shared FFN shard for one 512-token chunk."""
            hT = hpool.tile([P, FC, 512], DTR, tag="hT")
            for fc in range(FC):
                pg = ps_g.tile([P, 512], DT, tag="g")
                pu = ps_u.tile([P, 512], DT, tag="u")
                for dc in range(DC):
                    nc.tensor.matmul(pg[:], wg_sb[:, dc, fc * P:(fc + 1) * P],
                                     xT_sb[:, tc_i, dc],
                                     start=(dc == 0), stop=(dc == DC - 1))
                for dc in range(DC):
                    nc.tensor.matmul(pu[:], wu_sb[:, dc, fc * P:(fc + 1) * P],
                                     xT_sb[:, tc_i, dc],
                                     start=(dc == 0), stop=(dc == DC - 1))
                sg_act = act.tile([P, 512], DT, tag="silu")
                nc.scalar.activation(sg_act[:], pg[:], AF.Silu)
                nc.vector.tensor_mul(hT[:, fc], sg_act[:], pu[:])

            hsT = hpool.tile([P, SC, 512], DTR, tag="hsT")
            for sc in range(SC):
                pg = ps_g.tile([P, 512], DT, tag="g")
                pu = ps_u.tile([P, 512], DT, tag="u")
                for dc in range(DC):
                    nc.tensor.matmul(pg[:], sg_sb[:, dc, sc * P:(sc + 1) * P],
                                     xT_sb[:, tc_i, dc],
                                     start=(dc == 0), stop=(dc == DC - 1))
                for dc in range(DC):
                    nc.tensor.matmul(pu[:], su_sb[:, dc, sc * P:(sc + 1) * P],
                                     xT_sb[:, tc_i, dc],
                                     start=(dc == 0), stop=(dc == DC - 1))
                sg_act = act.tile([P, 512], DT, tag="silu")
                nc.scalar.activation(sg_act[:], pg[:], AF.Silu)
                nc.vector.tensor_mul(hsT[:, sc], sg_act[:], pu[:])

            # expert down + shared down, fused combine: o = y_e * cmb + y_s
            for j in range(4):
                tt = tc_i * 4 + j
                o_sb = outp.tile([P, D], DT, tag="o")
                for dn in range(2):
                    py1 = ps_y1.tile([P, 512], DT, tag="y1")
                    for fc in range(FC):
                        nc.tensor.matmul(py1[:], hT[:, fc, j * P:(j + 1) * P],
                                         wd_sb[:, fc, dn * 512:(dn + 1) * 512],
                                         start=(fc == 0), stop=(fc == FC - 1))
                    py2 = ps_y2.tile([P, 512], DT, tag="y2")
                    for sc in range(SC):
                        nc.tensor.matmul(py2[:], hsT[:, sc, j * P:(j + 1) * P],
                                         sd_sb[:, sc, dn * 512:(dn + 1) * 512],
                                         start=(sc == 0), stop=(sc == SC - 1))
                    rt = act.tile([P, 512], DT, tag="rt")
                    nc.vector.tensor_scalar(rt[:], py1[:], cmb_sb[:, tt:tt + 1],
                                            None, op0=ALU.mult)
                    nc.vector.scalar_tensor_tensor(
                        o_sb[:, dn * 512:(dn + 1) * 512], py2[:], 1.0, rt[:],
                        op0=ALU.mult, op1=ALU.add)
                nc.sync.dma_start(out[:, tt, :], o_sb[:])

        for tc_i in range(NTC):
            for j in range(4):
                router(tc_i * 4 + j)
            phase_b(tc_i)

    nc.compile()
    return nc


def _get_nc():
    global _NC_CACHE
    if _NC_CACHE is None:
        _NC_CACHE = _build_nc()
    return _NC_CACHE


def build_in_maps(inputs):
    x = np.asarray(inputs["hidden_states"], dtype=np.float32)
    # xT tiled [NTC, P, DC, 512]: element (tc, p, dc, t) = x[tc*512+t, dc*128+p]
    xt = np.ascontiguousarray(
        x.T.reshape(DC, P, NTC, 512).transpose(2, 1, 0, 3))
    rw = np.asarray(inputs["router_w"], dtype=np.float32)
    rwt = np.ascontiguousarray(rw.reshape(DC, P, E).transpose(1, 0, 2))
    eg = np.asarray(inputs["experts_gate"], dtype=np.float32)
    eu = np.asarray(inputs["experts_up"], dtype=np.float32)
    ed = np.asarray(inputs["experts_down"], dtype=np.float32)
    sgf = np.asarray(inputs["shared_gate"], dtype=np.float32)
    suf = np.asarray(inputs["shared_up"], dtype=np.float32)
    sdf = np.asarray(inputs["shared_down"], dtype=np.float32)

    def kxn(w):  # [K, N] -> [P, K/P, N] partition-major
        K, N = w.shape
        return np.ascontiguousarray(w.reshape(K // P, P, N).transpose(1, 0, 2))

    in_maps = []
    for c in range(NCORES):
        esel = np.zeros((P, E), dtype=np.float32)
        esel[:, c] = 1.0
        in_maps.append({
            "xt": xt,
            "rw": rwt,
            "wg": kxn(eg[c]),
            "wu": kxn(eu[c]),
            "wd": kxn(ed[c]),
            "sg": kxn(sgf[:, c * FS:(c + 1) * FS]),
            "su": kxn(suf[:, c * FS:(c + 1) * FS]),
            "sd": kxn(sdf[c * FS:(c + 1) * FS, :]),
            "esel": esel,
        })
    return in_maps


def kernel(hidden_states, router_w, experts_gate, experts_up, experts_down,
           shared_gate, shared_up, shared_down):
    nc = _get_nc()
    in_maps = build_in_maps({
        "hidden_states": hidden_states, "router_w": router_w,
        "experts_gate": experts_gate, "experts_up": experts_up,
        "experts_down": experts_down, "shared_gate": shared_gate,
        "shared_up": shared_up, "shared_down": shared_down,
    })
    res = run_bass_kernel_spmd(nc, in_maps, core_ids=list(range(NCORES)))
    # out is [P, TT, D] partition-major; unshard = sum partials + relayout
    acc = res.results[0]["out"].astype(np.float32)
    for c in range(1, NCORES):
        acc = acc + res.results[c]["out"]
    return np.ascontiguousarray(acc.transpose(1, 0, 2).reshape(T, D))


# revision 10
# speedup vs baseline: 1.3398x; 1.0328x over previous
"""MoE layer (8 experts, top-2, shared expert) on 8 Trainium2 cores.

Sharding: expert-parallel. Core c holds expert c's gate/up/down weights and
a 1/8 tensor-parallel shard (256 cols) of the shared FFN. x and the router
are replicated; every core computes the full router (exact fp32) and its
expert's SwiGLU densely over all tokens, scales by its combine column, adds
its shared-FFN partial, and returns a [T, D] partial. The host-side unshard
sums the 8 partials — exactly routed + shared of the reference.

All inputs are relaid out host-side during sharding so that every DMA is
contiguous per SBUF partition (128 big descriptors per transfer instead of
1024 small ones — HWDGE descriptor generation was the startup bottleneck).

Expert/shared matmuls run in f32r (full PE rate at moving-dim >= 256,
~1.5e-4 rel err). The router runs in true fp32: xT is DMA'd bit-exact into
an f32r-typed tile; dense matmuls consume it as f32r (the PE rounds
internally) while router matmuls read the same bits bitcast back to fp32.
The workload's minimum top2-vs-top3 logit gap (~3e-4) is too small for
f32r noise but is ~300x the fp32 matmul error.
"""

import numpy as np
from contextlib import ExitStack

import concourse.bass as bass
import concourse.tile as tile
from concourse import bacc, mybir
from concourse.bass_utils import run_bass_kernel_spmd

T, D, E = 2048, 1024, 8
F = 512          # per-expert FFN width
FS = 256         # shared FFN width per core (2048 / 8)
P = 128
NCORES = 8

TT = T // P      # 16 token tiles
DC = D // P      # 8 contraction chunks
FC = F // P      # 4 expert-f chunks
SC = FS // P     # 2 shared-f chunks
NTC = T // 512   # 4 token chunks of 512

DT = mybir.dt.float32
DTR = mybir.dt.float32r
AF = mybir.ActivationFunctionType
ALU = mybir.AluOpType
AX = mybir.AxisListType

_NC_CACHE = None


def _build_nc():
    nc = bacc.Bacc("TRN2", target_bir_lowering=False, debug=False,
                   num_devices=NCORES)
    # all inputs pre-relaid out host-side for partition-contiguous DMA
    xt = nc.dram_tensor("xt", [NTC, P, DC, 512], DT, kind="ExternalInput")
    rw = nc.dram_tensor("rw", [P, DC, E], DT, kind="ExternalInput")
    wg = nc.dram_tensor("wg", [P, DC, F], DT, kind="ExternalInput")
    wu = nc.dram_tensor("wu", [P, DC, F], DT, kind="ExternalInput")
    wd = nc.dram_tensor("wd", [P, FC, D], DT, kind="ExternalInput")
    sg = nc.dram_tensor("sg", [P, DC, FS], DT, kind="ExternalInput")
    su = nc.dram_tensor("su", [P, DC, FS], DT, kind="ExternalInput")
    sd = nc.dram_tensor("sd", [P, SC, D], DT, kind="ExternalInput")
    esel = nc.dram_tensor("esel", [P, E], DT, kind="ExternalInput")
    out = nc.dram_tensor("out", [P, TT, D], DT, kind="ExternalOutput")

    with tile.TileContext(nc) as tc, ExitStack() as ctx:
        const = ctx.enter_context(tc.tile_pool(name="const", bufs=1))
        esel_sb = const.tile([P, E], DT)
        nc.sync.dma_start(esel_sb[:], esel[:])
        rw_sb = const.tile([P, DC, E], DT)
        nc.sync.dma_start(rw_sb[:], rw[:])

        big = ctx.enter_context(tc.tile_pool(name="big", bufs=1))
        xT_sb = big.tile([P, NTC, DC, 512], DTR)  # transposed x (bit-exact fp32)
        cmb_sb = big.tile([P, TT], DT)            # combine column per token tile

        # All input DMAs on one HWDGE ring (FIFO) in just-in-time order:
        # xt chunk 0 lands first so the router/gate matmuls start early,
        # weights interleave behind it in the order phase_b consumes them.
        wgt = ctx.enter_context(tc.tile_pool(name="wgt", bufs=1))
        wg_sb = wgt.tile([P, DC, F], DTR)
        wu_sb = wgt.tile([P, DC, F], DTR)
        sg_sb = wgt.tile([P, DC, FS], DTR)
        su_sb = wgt.tile([P, DC, FS], DTR)
        wd_sb = wgt.tile([P, FC, D], DTR)
        sd_sb = wgt.tile([P, SC, D], DTR)

        nc.sync.dma_start(xT_sb[:, 0], xt[0].bitcast(DTR))
        nc.sync.dma_start(wg_sb[:], wg[:].bitcast(DTR))
        nc.sync.dma_start(wu_sb[:], wu[:].bitcast(DTR))
        nc.sync.dma_start(xT_sb[:, 1], xt[1].bitcast(DTR))
        nc.sync.dma_start(sg_sb[:], sg[:].bitcast(DTR))
        nc.sync.dma_start(su_sb[:], su[:].bitcast(DTR))
        nc.sync.dma_start(xT_sb[:, 2], xt[2].bitcast(DTR))
        nc.sync.dma_start(wd_sb[:], wd[:].bitcast(DTR))
        nc.sync.dma_start(sd_sb[:], sd[:].bitcast(DTR))
        nc.sync.dma_start(xT_sb[:, 3], xt[3].bitcast(DTR))

        pha = ctx.enter_context(tc.tile_pool(name="pha", bufs=2))
        act = ctx.enter_context(tc.tile_pool(name="act", bufs=2))
        hpool = ctx.enter_context(tc.tile_pool(name="hpool", bufs=2))
        outp = ctx.enter_context(tc.tile_pool(name="outp", bufs=3))

        # PSUM budget (8 banks): lg 2 + g 2 + u 2 + y1 1 + y2 1 = 8
        ps_r = ctx.enter_context(tc.tile_pool(name="ps_r", bufs=2, space="PSUM"))
        ps_g = ctx.enter_context(tc.tile_pool(name="ps_g", bufs=2, space="PSUM"))
        ps_u = ctx.enter_context(tc.tile_pool(name="ps_u", bufs=2, space="PSUM"))
        ps_y1 = ctx.enter_context(tc.tile_pool(name="ps_y1", bufs=1, space="PSUM"))
        ps_y2 = ctx.enter_context(tc.tile_pool(name="ps_y2", bufs=1, space="PSUM"))

        def router(tt):
            """Exact-fp32 router + top-2 combine weights for one token tile."""
            tci, j = divmod(tt, 4)
            csl = slice(j * P, (j + 1) * P)
            ps_lg = ps_r.tile([P, E], DT, tag="lg")
            for dc in range(DC):
                nc.tensor.matmul(ps_lg[:], xT_sb[:, tci, dc, csl].bitcast(DT),
                                 rw_sb[:, dc],
                                 start=(dc == 0), stop=(dc == DC - 1))

            m1 = pha.tile([P, 1], DT, tag="m1")
            nc.vector.reduce_max(out=m1[:], in_=ps_lg[:], axis=AX.X)
            nm1 = pha.tile([P, 1], DT, tag="nm1")
            nc.vector.tensor_scalar_mul(nm1[:], m1[:], -1.0)
            p_sb = pha.tile([P, E], DT, tag="p")
            nc.scalar.activation(p_sb[:], ps_lg[:], AF.Exp, bias=nm1[:])
            is1 = pha.tile([P, E], DT, tag="is1")
            nc.vector.tensor_scalar(is1[:], p_sb[:], 1.0, None, op0=ALU.is_ge)
            pm = pha.tile([P, E], DT, tag="pm")
            nc.vector.tensor_sub(pm[:], p_sb[:], is1[:])
            m2 = pha.tile([P, 1], DT, tag="m2")
            nc.vector.reduce_max(out=m2[:], in_=pm[:], axis=AX.X)
            s = pha.tile([P, 1], DT, tag="s")
            nc.vector.tensor_scalar_add(s[:], m2[:], 1.0)
            r = pha.tile([P, 1], DT, tag="r")
            nc.vector.reciprocal(r[:], s[:])
            t1 = pha.tile([P, E], DT, tag="t1")
            nc.vector.tensor_scalar(t1[:], p_sb[:], m2[:], r[:],
                                    op0=ALU.is_ge, op1=ALU.mult)
            w_sb = pha.tile([P, E], DT, tag="w")
            nc.vector.tensor_mul(w_sb[:], t1[:], p_sb[:])
            msk = pha.tile([P, E], DT, tag="msk")
            nc.vector.tensor_mul(msk[:], w_sb[:], esel_sb[:])
            nc.vector.reduce_sum(out=cmb_sb[:, tt:tt + 1], in_=msk[:], axis=AX.X)

        def phase_b(tc_i):
            """Expert SwiGLU + shared FFN shard for one 512-token chunk."""
            hT = hpool.tile([P, FC, 512], DTR, tag="hT")
            for fc in range(FC):
                pg = ps_g.tile([P, 512], DT, tag="g")
                pu = ps_u.tile([P, 512], DT, tag="u")
                for dc in range(DC):
                    nc.tensor.matmul(pg[:], wg_sb[:, dc, fc * P:(fc + 1) * P],
                                     xT_sb[:, tc_i, dc],
                                     start=(dc == 0), stop=(dc == DC - 1))
                for dc in range(DC):
                    nc.tensor.matmul(pu[:], wu_sb[:, dc, fc * P:(fc + 1) * P],
                                     xT_sb[:, tc_i, dc],
                                     start=(dc == 0), stop=(dc == DC - 1))
                sg_act = act.tile([P, 512], DT, tag="silu")
                nc.scalar.activation(sg_act[:], pg[:], AF.Silu)
                nc.vector.tensor_mul(hT[:, fc], sg_act[:], pu[:])

            hsT = hpool.tile([P, SC, 512], DTR, tag="hsT")
            for sc in range(SC):
                pg = ps_g.tile([P, 512], DT, tag="g")
                pu = ps_u.tile([P, 512], DT, tag="u")
                for dc in range(DC):
                    nc.tensor.matmul(pg[:], sg_sb[:, dc, sc * P:(sc + 1) * P],
                                     xT_sb[:, tc_i, dc],
                                     start=(dc == 0), stop=(dc == DC - 1))
                for dc in range(DC):
                    nc.tensor.matmul(pu[:], su_sb[:, dc, sc * P:(sc + 1) * P],
                                     xT_sb[:, tc_i, dc],
                                     start=(dc == 0), stop=(dc == DC - 1))
                sg_act = act.tile([P, 512], DT, tag="silu")
                nc.scalar.activation(sg_act[:], pg[:], AF.Silu)
                nc.vector.tensor_mul(hsT[:, sc], sg_act[:], pu[:])

            # expert down + shared down, fused combine: o = y_e * cmb + y_s
            for j in range(4):
                tt = tc_i * 4 + j
                o_sb = outp.tile([P, D], DT, tag="o")
                for dn in range(2):
                    py1 = ps_y1.tile([P, 512], DT, tag="y1")
                    for fc in range(FC):
                        nc.tensor.matmul(py1[:], hT[:, fc, j * P:(j + 1) * P],
                                         wd_sb[:, fc, dn * 512:(dn + 1) * 512],
                                         start=(fc == 0), stop=(fc == FC - 1))
                    py2 = ps_y2.tile([P, 512], DT, tag="y2")
                    for sc in range(SC):
                        nc.tensor.matmul(py2[:], hsT[:, sc, j * P:(j + 1) * P],
                                         sd_sb[:, sc, dn * 512:(dn + 1) * 512],
                                         start=(sc == 0), stop=(sc == SC - 1))
                    rt = act.tile([P, 512], DT, tag="rt")
                    nc.vector.tensor_scalar(rt[:], py1[:], cmb_sb[:, tt:tt + 1],
                                            None, op0=ALU.mult)
                    nc.vector.scalar_tensor_tensor(
                        o_sb[:, dn * 512:(dn + 1) * 512], py2[:], 1.0, rt[:],
                        op0=ALU.mult, op1=ALU.add)
                nc.sync.dma_start(out[:, tt, :], o_sb[:])

        for tc_i in range(NTC):
            for j in range(4):
                router(tc_i * 4 + j)
            phase_b(tc_i)

    nc.compile()
    return nc


def _get_nc():
    global _NC_CACHE
    if _NC_CACHE is None:
        _NC_CACHE = _build_nc()
    return _NC_CACHE


def build_in_maps(inputs):
    x = np.asarray(inputs["hidden_states"], dtype=np.float32)
    # xT tiled [NTC, P, DC, 512]: element (tc, p, dc, t) = x[tc*512+t, dc*128+p]
    xt = np.ascontiguousarray(
        x.T.reshape(DC, P, NTC, 512).transpose(2, 1, 0, 3))
    rw = np.asarray(inputs["router_w"], dtype=np.float32)
    rwt = np.ascontiguousarray(rw.reshape(DC, P, E).transpose(1, 0, 2))
    eg = np.asarray(inputs["experts_gate"], dtype=np.float32)
    eu = np.asarray(inputs["experts_up"], dtype=np.float32)
    ed = np.asarray(inputs["experts_down"], dtype=np.float32)
    sgf = np.asarray(inputs["shared_gate"], dtype=np.float32)
    suf = np.asarray(inputs["shared_up"], dtype=np.float32)
    sdf = np.asarray(inputs["shared_down"], dtype=np.float32)

    def kxn(w):  # [K, N] -> [P, K/P, N] partition-major
        K, N = w.shape
        return np.ascontiguousarray(w.reshape(K // P, P, N).transpose(1, 0, 2))

    in_maps = []
    for c in range(NCORES):
        esel = np.zeros((P, E), dtype=np.float32)
        esel[:, c] = 1.0
        in_maps.append({
            "xt": xt,
            "rw": rwt,
            "wg": kxn(eg[c]),
            "wu": kxn(eu[c]),
            "wd": kxn(ed[c]),
            "sg": kxn(sgf[:, c * FS:(c + 1) * FS]),
            "su": kxn(suf[:, c * FS:(c + 1) * FS]),
            "sd": kxn(sdf[c * FS:(c + 1) * FS, :]),
            "esel": esel,
        })
    return in_maps


def kernel(hidden_states, router_w, experts_gate, experts_up, experts_down,
           shared_gate, shared_up, shared_down):
    nc = _get_nc()
    in_maps = build_in_maps({
        "hidden_states": hidden_states, "router_w": router_w,
        "experts_gate": experts_gate, "experts_up": experts_up,
        "experts_down": experts_down, "shared_gate": shared_gate,
        "shared_up": shared_up, "shared_down": shared_down,
    })
    res = run_bass_kernel_spmd(nc, in_maps, core_ids=list(range(NCORES)))
    # out is [P, TT, D] partition-major; unshard = sum partials + relayout
    acc = res.results[0]["out"].astype(np.float32)
    for c in range(1, NCORES):
        acc = acc + res.results[c]["out"]
    return np.ascontiguousarray(acc.transpose(1, 0, 2).reshape(T, D))
